# revision 29
# baseline (speedup 1.0000x reference)
"""Trainium2 Bass kernel for nn_Block_46471546143558 (moe_routing).

Transformer block: h = x + Attn(LN1(x)); y = h + MoE(LN2(h)).
B=2, T=2048, D=1024, H=16 heads (hd=64), E=8 experts, top-2, cap=640.

Sharding over 8 NeuronCores:
  - attention: head-parallel (2 heads/core); LN/proj/residual token-parallel
    (each core owns a contiguous 512-token shard of the flattened 4096)
  - MoE: expert-parallel (1 expert/core), routing replicated from an
    AllGather of fp32 gate logits
  - collectives: AG(x_ln bf16) -> A2A(attn heads->token shards, fp32
    unnormalized + per-head denominators) -> AG(logits fp32) +
    AG(moe_in fp8) -> AG(expert outputs fp8).

Precision: the routing (top-2 + capacity cumsum) is knife-edge sensitive
(min top2/top3 logit gap ~2e-4), so the path into the logits is kept at
>=fp22 effective precision: probs fp32, PV fp32r, attention payload fp32,
per-head normalization after transfer, proj fp32r, gate matmul fp32.
The MoE value path is insensitive and runs in fp8 (weights pre-scaled
x64 to clear the e4m3 subnormal floor; compensation folded into the
gelu activation scale and the bias add).
"""

import math
import sys

sys.path.insert(0, "/opt/trn_rl_repo")

import ml_dtypes
import numpy as np

import concourse.bass as bass
import concourse.mybir as mybir
import concourse.tile as tile
from concourse import bacc
from concourse.bass import IndirectOffsetOnAxis
from concourse.bass_utils import run_bass_kernel_spmd
from concourse.tile import add_dep_helper

B, T, D, H, E, K = 2, 2048, 1024, 16, 8, 2
HD = D // H  # 64
N = B * T  # 4096
CAP = math.ceil(1.25 * N / E)  # 640
NC = 8
SH = N // NC  # 512
NT = N // 128  # 32
ST = CAP // 128  # 5
F1 = 4 * D  # 4096

FP = mybir.dt.float32
FPR = mybir.dt.float32r
BF = mybir.dt.bfloat16
F8 = mybir.dt.float8e4
F16 = mybir.dt.float16
I32 = mybir.dt.int32
ATT = BF  # attention qkv matmul dtype
NPATT = ml_dtypes.bfloat16
NPF8 = ml_dtypes.float8_e4m3

WS = 64.0  # fp8 weight prescale for w1/w2 (clears e4m3 subnormal floor)

OOB = 1.0e9
AF = mybir.ActivationFunctionType
AL = mybir.AluOpType
DR = mybir.MatmulPerfMode.DoubleRow

PAY = 2 * (HD + 1)  # 130 rows per A2A block: [h0 out 64][h0 den][h1 out 64][h1 den]


def build(debug=False):
    nc = bacc.Bacc("TRN2", target_bir_lowering=False, debug=False,
                   enable_asserts=True, num_devices=NC)

    def din(name, shape, dt):
        return nc.dram_tensor(name, list(shape), dt, kind="ExternalInput").ap()

    xT = din("xT", [128, 8, SH], FP)
    qk_w = din("qk_w", [128, 8, 256], ATT)
    v_w = din("v_w", [128, 8, 128], ATT)
    proj_w = din("proj_w", [128, 8, 1024], FPR)
    gate_w = din("gate_w", [128, 8, 8], FP)
    Ctab = din("Ctab", [128, NT, 128], BF)
    Stab = din("Stab", [128, NT, 128], BF)
    dmask = din("dmask", [4, 128, 512], BF)
    identb = din("identb", [128, 128], BF)
    identf = din("identf", [128, 128], FP)
    tri = din("tri", [128, 128], BF)
    tri32 = din("tri32", [32, 32], FP)
    ones32_128 = din("ones32_128", [32, 128], BF)
    onesf = din("onesf", [128, 1], FP)
    onesb = din("onesb", [128, 1], BF)
    ones_r_f = din("ones_r_f", [1, 128], FP)
    sel16 = din("sel16", [16, 8, 128], FPR)
    ln1g = din("ln1g", [128, 8], FP)
    ln1b = din("ln1b", [128, 8], FP)
    cc1 = din("cc1", [128, 8], FP)
    cc2 = din("cc2", [128, 8], FP)
    base8 = din("base8", [128, 8], FP)
    esel = din("esel", [128, 8], FP)
    shsel = din("shsel", [128, 4, NT], FP)
    tid1 = din("tid1", [128, NT], I32)
    w1l = din("w1l", [1024, F1], F8)
    b1l = din("b1l", [128, 32], FP)
    w2l = din("w2l", [F1, 1024], F8)
    b2l = din("b2l", [128, 1024], FP)

    y_out = nc.dram_tensor("y", [SH, D], FP, kind="ExternalOutput").ap()
    dbg = {}
    if debug:
        def dout(name, shape, dt=FP):
            return nc.dram_tensor(name, list(shape), dt, kind="ExternalOutput").ap()
        dbg["xln"] = dout("dbg_xln", [128, 8, SH])
        dbg["h"] = dout("dbg_h", [128, 4, D])
        dbg["logits"] = dout("dbg_logits", [128, 4, 8])
        dbg["pos"] = dout("dbg_pos", [128, NT, 8])
        dbg["gidx"] = dout("dbg_gidx", [128, ST], I32)
        dbg["cidx"] = dout("dbg_cidx", [128, 4, 2], I32)
        dbg["g12"] = dout("dbg_g12", [128, 4, 2])
        dbg["eo"] = dout("dbg_eo", [CAP, D], BF)

    tables = nc.dram_tensor("tbls", [8, 768, 1], I32, kind="Internal").ap()
    ag2_out = nc.dram_tensor("ag2o", [NC * PAY, SH], FPR, kind="Internal").ap()
    ag3b_out = nc.dram_tensor("ag3bo", [N, D], F8, kind="Internal",
                              addr_space="Shared").ap()
    ag4_out = nc.dram_tensor("ag4o", [NC * CAP, D], F8, kind="Internal",
                             addr_space="Shared").ap()
    RG = [list(range(NC))]

    with tile.TileContext(nc) as tc:
        with tc.tile_pool(name="dram", bufs=1, space="DRAM") as dram, \
             tc.tile_pool(name="pers", bufs=1) as pers, \
             tc.tile_pool(name="const", bufs=1) as const:
            ag1_in = dram.tile([128, 8, SH], ATT)
            ag1_out = dram.tile([NC, 128, 8, SH], ATT, addr_space="Shared")
            ag2_in = dram.tile([NC * PAY, SH], FPR)
            ag3a_in = dram.tile([SH, 8], FP)
            ag3a_out = dram.tile([N, 8], FP, addr_space="Shared")
            ag3b_in = dram.tile([SH, D], F8)
            ag4_in = dram.tile([CAP, D], F8)

            qh0 = pers.tile([64, N], ATT)
            qh1 = pers.tile([64, N], ATT)
            kh0 = pers.tile([64, N], ATT)
            kh1 = pers.tile([64, N], ATT)
            v_tm = pers.tile([128, NT, 256], F16)
            xT_sb = pers.tile([128, 8, SH], FP)
            h_sb = pers.tile([128, 4, D], FP)
            m2c = pers.tile([128, 4], FP)
            r2c = pers.tile([128, 4], FP)
            lg_my = pers.tile([128, 4, 8], FP)
            g12 = pers.tile([128, 4, 2], FP)
            cidx = pers.tile([128, 4, 2], I32)
            gidx = pers.tile([128, ST], I32)

            qkw_sb = const.tile([128, 8, 256], ATT)
            vw_sb = const.tile([128, 8, 128], ATT)
            identb_sb = const.tile([128, 128], BF)
            identf_sb = const.tile([128, 128], FP)
            tri_sb = const.tile([128, 128], BF)
            tri32_sb = const.tile([32, 32], FP)
            o32_sb = const.tile([32, 128], BF)
            onesf_sb = const.tile([128, 1], FP)
            onesb_sb = const.tile([128, 1], BF)
            orf_sb = const.tile([1, 128], FP)
            sel16_sb = const.tile([16, 8, 128], FPR)
            ln1g_sb = const.tile([128, 8], FP)
            ln1b_sb = const.tile([128, 8], FP)
            cc1_sb = const.tile([128, 8], FP)
            cc2_sb = const.tile([128, 8], FP)
            base8_sb = const.tile([128, 8], FP)
            esel_sb = const.tile([128, 8], FP)
            shsel_sb = const.tile([128, 4, NT], FP)
            tid1_sb = const.tile([128, NT], I32)
            gatew_sb = const.tile([128, 8, 8], FP)
            b1l_sb = const.tile([128, 32], FP)
            b2l_sb = const.tile([128, 1024], FP)
            eps_sb = const.tile([128, 1], FP)
            inv64_sb = const.tile([128, 1], FP)
            nc.vector.memset(eps_sb[:], 1e-5)
            nc.vector.memset(inv64_sb[:], 1.0 / WS)
            dmask_sb = const.tile([128, 4, 512], BF)
            nc.sync.dma_start(xT_sb[:], xT[:])
            for sb, dr in [(qkw_sb, qk_w), (vw_sb, v_w), (identb_sb, identb),
                           (identf_sb, identf), (tri_sb, tri), (tri32_sb, tri32),
                           (o32_sb, ones32_128), (onesf_sb, onesf),
                           (onesb_sb, onesb), (orf_sb, ones_r_f),
                           (sel16_sb, sel16), (ln1g_sb, ln1g), (ln1b_sb, ln1b),
                           (cc1_sb, cc1), (cc2_sb, cc2), (base8_sb, base8),
                           (esel_sb, esel), (shsel_sb, shsel), (tid1_sb, tid1),
                           (gatew_sb, gate_w), (b1l_sb, b1l), (b2l_sb, b2l)]:
                nc.sync.dma_start(sb[:], dr[:])
            for q in range(4):
                nc.sync.dma_start(dmask_sb[:, q], dmask[q])

            # ======== P1: LN1 (feature-major) + AG1 ========
            with nc.named_scope("P1"), \
                 tc.tile_pool(name="p1", bufs=1) as p1, \
                 tc.tile_pool(name="p1ps", bufs=2, space="PSUM") as p1ps:
                sq = p1.tile([128, 8, SH], FP)
                nc.scalar.square(sq[:], xT_sb[:])
                msum = p1ps.tile([1, SH], FP, tag="ms")
                ssum = p1ps.tile([1, SH], FP, tag="ss")
                for o in range(8):
                    nc.tensor.matmul(msum[:], onesf_sb[:], xT_sb[:, o],
                                     start=(o == 0), stop=(o == 7))
                for o in range(8):
                    nc.tensor.matmul(ssum[:], onesf_sb[:], sq[:, o],
                                     start=(o == 0), stop=(o == 7))
                mrow = p1.tile([1, SH], FP)
                rrow = p1.tile([1, SH], FP)
                nc.scalar.mul(mrow[:], msum[:], 1.0 / D)
                nc.scalar.mul(rrow[:], ssum[:], 1.0 / D)
                m2row = p1.tile([1, SH], FP)
                nc.vector.tensor_mul(m2row[:], mrow[:], mrow[:])
                nc.vector.tensor_sub(rrow[:], rrow[:], m2row[:])
                nc.scalar.activation(rrow[:], rrow[:], AF.Sqrt, bias=eps_sb[0:1, 0:1])
                nc.vector.reciprocal(rrow[:], rrow[:])
                mb = p1ps.tile([128, SH], FP, tag="bc")
                rb = p1ps.tile([128, SH], FP, tag="bc")
                nc.tensor.matmul(mb[:], orf_sb[:], mrow[:], start=True, stop=True)
                nc.tensor.matmul(rb[:], orf_sb[:], rrow[:], start=True, stop=True)
                xln = p1.tile([128, 8, SH], ATT)
                for o in range(8):
                    t1 = p1.tile([128, SH], FP, tag="t1")
                    nc.vector.tensor_sub(t1[:], xT_sb[:, o], mb[:])
                    nc.vector.tensor_mul(t1[:], t1[:], rb[:])
                    nc.vector.tensor_scalar(xln[:, o], t1[:],
                                            ln1g_sb[:, o:o + 1], ln1b_sb[:, o:o + 1],
                                            AL.mult, AL.add)
                nc.sync.dma_start(ag1_in[:], xln[:])
                if debug:
                    xlnf = p1.tile([128, 8, SH], FP)
                    nc.vector.tensor_copy(xlnf[:], xln[:])
                    nc.sync.dma_start(dbg["xln"][:], xlnf[:])
            with nc.named_scope("cc1"):
                nc.gpsimd.collective_compute("AllGather", AL.bypass, RG,
                                             ins=[ag1_in.opt()], outs=[ag1_out.opt()])

            # ======== P2: QKV + RoPE + V token-major ========
            with nc.named_scope("P2"), \
                 tc.tile_pool(name="p2", bufs=2) as p2, \
                 tc.tile_pool(name="p2c", bufs=1) as p2c, \
                 tc.tile_pool(name="p2ps", bufs=2, space="PSUM") as p2ps, \
                 tc.tile_pool(name="p2v", bufs=2, space="PSUM") as p2v:
                ct_sb = p2c.tile([128, NT, 128], BF)
                st_sb = p2c.tile([128, NT, 128], BF)
                nc.sync.dma_start(ct_sb[:], Ctab[:])
                nc.sync.dma_start(st_sb[:], Stab[:])
                nc.vector.memset(v_tm[:], 0.0)
                nc.vector.memset(v_tm[:, :, 0:1], 1.0)
                nc.vector.memset(v_tm[:, :, 128:129], 1.0)
                v_tm4 = v_tm.rearrange("p t (a b) -> p t a b", a=2)
                for s in range(NC):
                    xl = p2.tile([128, 8, SH], ATT, tag="xl")
                    nc.sync.dma_start(xl[:], ag1_out[s])
                    for mt in range(2):
                        qdst = qh0 if mt == 0 else qh1
                        kdst = kh0 if mt == 0 else kh1
                        ps = p2ps.tile([128, SH], FP, tag=f"qk{mt}", bufs=1)
                        for o in range(8):
                            nc.tensor.matmul(ps[:], qkw_sb[:, o, 128 * mt:128 * mt + 128],
                                             xl[:, o], start=(o == 0), stop=(o == 7))
                        Cv = ct_sb.rearrange("p t f -> p (t f)")[:, SH * s:SH * (s + 1)]
                        Sv = st_sb.rearrange("p t f -> p (t f)")[:, SH * s:SH * (s + 1)]
                        psb = p2.tile([128, SH], BF, tag="psb")
                        nc.vector.tensor_copy(psb[:], ps[:])
                        tmp = p2.tile([128, SH], BF, tag="tmp")
                        acc = p2.tile([128, SH], BF, tag="acc")
                        nc.vector.tensor_mul(tmp[32:64], psb[0:32], Sv[0:32])
                        nc.vector.tensor_mul(tmp[0:32], psb[32:64], Sv[32:64])
                        nc.vector.tensor_mul(tmp[96:128], psb[64:96], Sv[64:96])
                        nc.vector.tensor_mul(tmp[64:96], psb[96:128], Sv[96:128])
                        nc.vector.tensor_mul(acc[:], psb[:], Cv[:])
                        nc.vector.tensor_add(qdst[:, SH * s:SH * (s + 1)],
                                             acc[0:64], tmp[0:64])
                        nc.vector.tensor_add(kdst[:, SH * s:SH * (s + 1)],
                                             acc[64:128], tmp[64:128])
                    for tt in range(4):
                        vps = p2v.tile([128, 128], FP, tag="v")
                        for o in range(8):
                            nc.tensor.matmul(vps[:], xl[:, o, 128 * tt:128 * tt + 128],
                                             vw_sb[:, o], start=(o == 0), stop=(o == 7))
                        g = 4 * s + tt
                        nc.vector.tensor_copy(
                            v_tm4[:, g, :, 64:128],
                            vps.rearrange("p (a b) -> p a b", a=2))

            # ======== P3: scores + softmax + PV (unnormalized); A2A ========
            with nc.named_scope("P3"), \
                 tc.tile_pool(name="p3", bufs=3) as p3, \
                 tc.tile_pool(name="p3ps", bufs=2, space="PSUM") as p3ps, \
                 tc.tile_pool(name="p3po", bufs=2, space="PSUM") as p3po:
                a2v = ag2_in.rearrange("(k r) t -> k r t", r=PAY)
                for hh in range(2):
                    qh = qh0 if hh == 0 else qh1
                    kh = kh0 if hh == 0 else kh1
                    for qb in range(8):
                        seq, qbl = qb // 4, qb % 4
                        kts = list(range(16 * seq, 16 * seq + 4 * qbl + 4))
                        ov = p3po.tile([128, 512], FP, tag="ov")
                        for i, kt in enumerate(kts):
                            sc = p3ps.tile([128, 512], FP, tag="sc")
                            nc.tensor.matmul(sc[:], kh[:, 128 * kt:128 * kt + 128],
                                             qh[:, 512 * qb:512 * (qb + 1)],
                                             start=True, stop=True)
                            ex = p3.tile([128, 512], F16, tag="ex")
                            nc.scalar.activation(ex[:], sc[:], AF.Exp,
                                                 scale=1.0 / math.sqrt(HD))
                            loc = kt - 16 * seq - 4 * qbl
                            if loc >= 0:
                                nc.vector.tensor_mul(ex[:], ex[:], dmask_sb[:, loc])
                            nc.tensor.matmul(ov[:], v_tm[:, kt, 128 * hh:128 * hh + 128],
                                             ex[:],
                                             start=(i == 0), stop=(i == len(kts) - 1))
                        av = p3.tile([65, 512], FPR, tag="av")
                        nc.vector.tensor_copy(av[0:64], ov[64:128])
                        nc.vector.tensor_copy(av[64:65], ov[0:1])
                        r0 = 65 * hh
                        nc.sync.dma_start(a2v[qb, r0:r0 + 65], av[:])
            with nc.named_scope("cc2"):
                nc.gpsimd.collective_compute("AllToAll", AL.bypass, RG,
                                             ins=[ag2_in.opt()], outs=[ag2_out.opt()])

            # ======== P4: normalize + proj + h + LN2 + logits + moe_in ========
            with nc.named_scope("P4"), \
                 tc.tile_pool(name="p4", bufs=1) as p4, \
                 tc.tile_pool(name="p4ps", bufs=2, space="PSUM") as p4ps, \
                 tc.tile_pool(name="p4pt", bufs=1, space="PSUM") as p4pt:
                o2v = ag2_out.rearrange("(k r) t -> k r t", r=PAY)
                o2d = ag2_out.rearrange("(k j s) t -> k j s t", j=2, s=HD + 1)
                myc = p4.tile([128, 8, SH], FPR)
                nc.sync.dma_start(myc[0:64], o2v[:, 0:64].rearrange("k r t -> r k t"))
                nc.sync.dma_start(myc[64:128], o2v[:, 65:129].rearrange("k r t -> r k t"))
                den16 = p4.tile([16, SH], FPR)
                nc.sync.dma_start(den16[:], o2d[:, :, 64].rearrange("k j t -> (k j) t"))
                with nc.allow_low_precision(reason="fp32r recip feeds fp32r matmul"):
                    nc.vector.reciprocal(den16[:], den16[:])
                hT = p4.tile([128, 8, SH], FP)
                pw = p4.tile([128, 8, 1024], FPR)
                nc.sync.dma_start(pw[:], proj_w[:])
                for ki in range(8):
                    rbp = p4pt.tile([128, SH], FP, tag="rb")
                    nc.tensor.matmul(rbp[:], sel16_sb[:, ki],
                                     den16[:], start=True, stop=True)
                    nc.vector.tensor_mul(myc[:, ki], myc[:, ki], rbp[:])
                for do in range(8):
                    pp = p4ps.tile([128, SH], FP, tag="pp")
                    for ki in range(8):
                        nc.tensor.matmul(pp[:], pw[:, ki, 128 * do:128 * do + 128],
                                         myc[:, ki],
                                         start=(ki == 0), stop=(ki == 7))
                    nc.vector.tensor_add(hT[:, do], pp[:], xT_sb[:, do])
                    for tt in range(4):
                        tp = p4pt.tile([128, 128], FP, tag="tp", bufs=2)
                        nc.tensor.transpose(tp[:], hT[:, do, 128 * tt:128 * tt + 128],
                                            identf_sb[:])
                        nc.vector.tensor_copy(h_sb[:, tt, 128 * do:128 * do + 128], tp[:])
                scr = p4.tile([128, D], FP, tag="scr")
                sqc = p4.tile([128, 4], FP, tag="sqc")
                for tt in range(4):
                    nc.vector.tensor_reduce(m2c[:, tt:tt + 1], h_sb[:, tt],
                                            axis=mybir.AxisListType.X, op=AL.add)
                    nc.scalar.activation(scr[:], h_sb[:, tt], AF.Square,
                                         accum_out=sqc[:, tt:tt + 1])
                nc.vector.tensor_scalar_mul(m2c[:], m2c[:], 1.0 / D)
                nc.vector.tensor_scalar_mul(sqc[:], sqc[:], 1.0 / D)
                vv = p4.tile([128, 4], FP, tag="vv")
                nc.vector.tensor_mul(vv[:], m2c[:], m2c[:])
                nc.vector.tensor_sub(vv[:], sqc[:], vv[:])
                nc.scalar.activation(vv[:], vv[:], AF.Sqrt, bias=eps_sb[:, 0:1])
                nc.vector.reciprocal(r2c[:], vv[:])
                lps = p4pt.tile([8, SH], FP, tag="lps")
                for ki in range(8):
                    nc.tensor.matmul(lps[:], gatew_sb[:, ki], hT[:, ki],
                                     start=(ki == 0), stop=(ki == 7))
                lsb = p4.tile([8, SH], FP, tag="lsb")
                nc.vector.tensor_copy(lsb[:], lps[:])
                rm = p4.tile([128, 4], FP, tag="rm")
                nc.vector.tensor_mul(rm[:], r2c[:], m2c[:])
                t8 = p4.tile([128, 8], FP, tag="t8")
                for tt in range(4):
                    ltp = p4pt.tile([128, 8], FP, tag="ltp")
                    nc.tensor.transpose(ltp[:], lsb[0:8, 128 * tt:128 * tt + 128],
                                        identf_sb[0:8, 0:8])
                    nc.vector.tensor_scalar_mul(lg_my[:, tt], ltp[:], r2c[:, tt:tt + 1])
                    nc.vector.tensor_scalar_mul(t8[:], cc1_sb[:], rm[:, tt:tt + 1])
                    nc.vector.tensor_sub(lg_my[:, tt], lg_my[:, tt], t8[:])
                    nc.vector.tensor_add(lg_my[:, tt], lg_my[:, tt], cc2_sb[:])
                nc.sync.dma_start(ag3a_in.rearrange("(t p) e -> p t e", p=128), lg_my[:])
                moein = p4.tile([128, 4, D], F8)
                for tt in range(4):
                    nc.vector.tensor_scalar(moein[:, tt], h_sb[:, tt],
                                            m2c[:, tt:tt + 1], r2c[:, tt:tt + 1],
                                            AL.subtract, AL.mult)
                nc.sync.dma_start(ag3b_in.rearrange("(t p) d -> p t d", p=128), moein[:])
                if debug:
                    nc.sync.dma_start(dbg["h"][:], h_sb[:])
                    nc.sync.dma_start(dbg["logits"][:], lg_my[:])
            with nc.named_scope("cc3a"):
                cc3a_i = nc.gpsimd.collective_compute(
                    "AllGather", AL.bypass, RG,
                    ins=[ag3a_in.opt()], outs=[ag3a_out.opt()])
            with nc.named_scope("cc3b"):
                cc3b_i = nc.gpsimd.collective_compute(
                    "AllGather", AL.bypass, RG,
                    ins=[ag3b_in.opt()], outs=[ag3b_out.opt()])
                add_dep_helper(cc3b_i.ins, cc3a_i.ins, sync=True,
                               reason="AG3a (routing) before AG3b on CC core")

            # ======== P5: replicated routing ========
            with nc.named_scope("P5"), \
                 tc.tile_pool(name="p5", bufs=2) as p5, \
                 tc.tile_pool(name="p5ps", bufs=2, space="PSUM") as p5ps:
                lg4 = p5.tile([128, NC, 4, 8], FP)
                nc.sync.dma_start(lg4[:], ag3a_out.rearrange("(r t p) e -> p r t e",
                                                             p=128, t=4))
                lg_all = lg4.rearrange("p r t e -> p (r t) e")
                mask = p5.tile([128, NT, 8], BF)
                v8 = p5.tile([128, 8], FP, tag="v8")
                m1 = p5.tile([128, NT, 1], FP, tag="m1")
                mask0 = p5.tile([128, NT, 8], FP, tag="mask0")
                lg2 = p5.tile([128, NT, 8], FP, tag="lg2")
                nc.vector.tensor_reduce(m1[:], lg_all[:],
                                        axis=mybir.AxisListType.X, op=AL.max)
                nc.vector.tensor_tensor(mask0[:], lg_all[:],
                                        m1[:].to_broadcast([128, NT, 8]), AL.is_ge)
                nc.vector.tensor_scalar(lg2[:], mask0[:], -OOB, None, AL.mult)
                nc.vector.tensor_add(lg2[:], lg2[:], lg_all[:])
                nc.vector.tensor_reduce(m1[:], lg2[:],
                                        axis=mybir.AxisListType.X, op=AL.max)
                nc.vector.tensor_tensor(mask[:], lg_all[:],
                                        m1[:].to_broadcast([128, NT, 8]), AL.is_ge)
                trow_ps = p5ps.tile([1, NT * 8], FP, tag="bc")
                nc.tensor.matmul(trow_ps[:], onesb_sb[:],
                                 mask.rearrange("p t e -> p (t e)"), start=True, stop=True)
                trow = p5.tile([1, NT * 8], BF, tag="trow")
                nc.vector.tensor_copy(trow[:], trow_ps[:])
                totd = dram.tile([NT, 8], BF)
                nc.sync.dma_start(totd.rearrange("t e -> (t e)")[None, :], trow[:])
                tot32 = p5.tile([32, 8], BF, tag="tot32")
                nc.sync.dma_start(tot32[:], totd[:])
                rhs2 = p5.tile([32, NT, 8], BF, tag="rhs2")
                nc.vector.tensor_tensor(rhs2[:],
                                        tot32[:, None, :].to_broadcast([32, NT, 8]),
                                        tri32_sb[:, :, None].to_broadcast([32, NT, 8]),
                                        AL.mult)
                pos_ps = p5ps.tile([128, NT * 8], FP, tag="pos")
                nc.tensor.matmul(pos_ps[:], tri_sb[:], mask.rearrange("p t e -> p (t e)"),
                                 start=True, stop=False)
                nc.tensor.matmul(pos_ps[:], o32_sb[:], rhs2.rearrange("p t e -> p (t e)"),
                                 start=False, stop=True)
                pos = p5.tile([128, NT, 8], FP)
                nc.vector.tensor_copy(pos[:], pos_ps.rearrange("p (t e) -> p t e", e=8))
                if debug:
                    nc.sync.dma_start(dbg["pos"][:], pos[:])
                # dispatch offsets for my expert
                pe = p5.tile([128, NT], FP, tag="pe")
                me = p5.tile([128, NT], FP, tag="me")
                scr8 = p5.tile([128, 8], FP, tag="scr8")
                eb = p5.tile([128, NT, 8], FP, tag="eb")
                esb = esel_sb[:, None, :].to_broadcast([128, NT, 8])
                nc.vector.tensor_tensor(eb[:], pos[:], esb, AL.mult)
                nc.vector.tensor_reduce(pe[:, :, None], eb[:],
                                        axis=mybir.AxisListType.X, op=AL.add)
                nc.vector.tensor_tensor(eb[:], mask[:], esb, AL.mult)
                nc.vector.tensor_reduce(me[:, :, None], eb[:],
                                        axis=mybir.AxisListType.X, op=AL.add)
                offf = p5.tile([128, NT], FP, tag="offf")
                nc.vector.tensor_scalar(offf[:], me[:], -OOB, OOB, AL.mult, AL.add)
                nc.vector.tensor_add(offf[:], offf[:], pe[:])
                nc.vector.tensor_scalar_add(offf[:], offf[:], -1.0)
                offi = p5.tile([128, NT], I32, tag="offi")
                nc.vector.tensor_copy(offi[:], offf[:])
                ztab = p5.tile([128, 48], I32, tag="ztab")
                nc.vector.memset(ztab[:], 0)
                zti = nc.sync.dma_start(
                    tables.rearrange("g (o p) f -> p (g o) f", p=128),
                    ztab[:, :, None])
                tflat = tables.rearrange("g r f -> (g r) f")
                scat = []
                for g in range(NT):
                    sci = nc.gpsimd.indirect_dma_start(
                        out=tflat,
                        out_offset=IndirectOffsetOnAxis(ap=offi[:, g:g + 1], axis=0),
                        in_=tid1_sb[:, g:g + 1],
                        in_offset=None, element_offset=(g // 4) * 768,
                        bounds_check=CAP - 1, oob_is_err=False)
                    add_dep_helper(sci.ins, zti.ins, sync=True,
                                   reason="scatter after table zeroing")
                    scat.append(sci)
                tpart = p5.tile([128, 8, 6], I32, tag="tpart")
                tpi = nc.sync.dma_start(
                    tpart[:],
                    tables.rearrange("g (o p) f -> p g (o f)", p=128))
                for sci in scat:
                    add_dep_helper(tpi.ins, sci.ins, sync=True,
                                   reason="table readback after scatters")
                tcomb = p5.tile([128, 6], I32, tag="tcomb")
                nc.vector.tensor_add(tcomb[:], tpart[:, 0], tpart[:, 1])
                for g in range(2, 8):
                    nc.vector.tensor_add(tcomb[:], tcomb[:], tpart[:, g])
                nc.vector.tensor_scalar_add(gidx[:], tcomb[:, 0:ST], -1)
                if debug:
                    nc.sync.dma_start(dbg["gidx"][:], gidx[:])
                # my combine indices + gate values
                myp = p5.tile([128, 4, 8], FP, tag="myp")
                tmpb = p5.tile([128, NT, 8], FP, tag="tmpb")
                for tt in range(4):
                    nc.vector.tensor_tensor(tmpb[:], pos[:],
                                            shsel_sb[:, tt, :, None].to_broadcast(
                                                [128, NT, 8]), AL.mult)
                    nc.vector.tensor_reduce(myp[:, tt, :, None],
                                            tmpb.rearrange("p t e -> p e t"),
                                            axis=mybir.AxisListType.X, op=AL.add)
                oh0 = p5.tile([128, 8], FP, tag="oh0")
                oh1 = p5.tile([128, 8], FP, tag="oh1")
                ex8 = p5.tile([128, 8], FP, tag="ex8")
                den8 = p5.tile([128, 1], FP, tag="den8")
                v12 = p5.tile([128, 2], FP, tag="v12")
                fl = p5.tile([128, 1], FP, tag="fl")
                sl = p5.tile([128, 1], FP, tag="sl")
                kf = p5.tile([128, 1], FP, tag="kf")
                tof = p5.tile([128, 1], FP, tag="tof")
                cidf = p5.tile([128, 4, 2], FP, tag="cidf")
                fb = p5.tile([128, 8], FP, tag="fb")
                for tt in range(4):
                    nc.vector.max(v8[:], lg_my[:, tt])
                    nc.vector.tensor_copy(v12[:], v8[:, 0:2])
                    nc.scalar.activation(ex8[:], lg_my[:, tt], AF.Exp, accum_out=den8[:])
                    nc.vector.reciprocal(den8[:], den8[:])
                    nc.scalar.activation(g12[:, tt], v12[:], AF.Exp)
                    nc.vector.tensor_scalar_mul(g12[:, tt], g12[:, tt], den8[:])
                    nc.vector.tensor_add(fb[:], myp[:, tt], base8_sb[:])
                    nc.vector.tensor_scalar(oh0[:], lg_my[:, tt], v8[:, 0:1], None, AL.is_ge)
                    nc.vector.tensor_scalar(oh1[:], lg_my[:, tt], v8[:, 1:2], None, AL.is_ge)
                    nc.vector.tensor_sub(oh1[:], oh1[:], oh0[:])
                    for kk, oh in ((0, oh0), (1, oh1)):
                        nc.vector.tensor_mul(scr8[:], myp[:, tt], oh[:])
                        nc.vector.tensor_reduce(sl[:], scr8[:],
                                                axis=mybir.AxisListType.X, op=AL.add)
                        nc.vector.tensor_mul(scr8[:], fb[:], oh[:])
                        nc.vector.tensor_reduce(fl[:], scr8[:],
                                                axis=mybir.AxisListType.X, op=AL.add)
                        nc.vector.tensor_scalar(kf[:], sl[:], CAP + 0.5, None, AL.is_le)
                        nc.vector.tensor_scalar(tof[:], kf[:], -OOB, OOB, AL.mult, AL.add)
                        nc.vector.scalar_tensor_tensor(cidf[:, tt, kk:kk + 1], fl[:],
                                                       kf[:, 0:1], tof[:],
                                                       AL.mult, AL.add)
                nc.vector.tensor_scalar_add(cidf[:], cidf[:], -1.0)
                nc.vector.tensor_copy(cidx[:], cidf[:])
                if debug:
                    nc.sync.dma_start(dbg["cidx"][:], cidx[:])
                    nc.sync.dma_start(dbg["g12"][:], g12[:])

            # ======== P6: expert MLP (fp8, DoubleRow); AG4 ========
            with nc.named_scope("P6"), \
                 tc.tile_pool(name="p6", bufs=1) as p6, \
                 tc.tile_pool(name="p6g", bufs=2) as p6g, \
                 tc.tile_pool(name="p6w", bufs=2) as p6w, \
                 tc.tile_pool(name="p6w2", bufs=1) as p6w2, \
                 tc.tile_pool(name="p6ps", bufs=2, space="PSUM") as p6ps, \
                 tc.tile_pool(name="p6pt", bufs=2, space="PSUM") as p6pt:
                bufT = p6.tile([128, 8, CAP], F8)
                for j in range(ST):
                    gb = p6g.tile([128, D], F8, tag="gb")
                    nc.vector.memset(gb[:], 0.0)
                    gbi = nc.gpsimd.indirect_dma_start(
                        out=gb[:], out_offset=None, in_=ag3b_out[:],
                        in_offset=IndirectOffsetOnAxis(ap=gidx[:, j:j + 1], axis=0),
                        bounds_check=N - 1, oob_is_err=False)
                    add_dep_helper(gbi.ins, cc3b_i.ins, sync=True,
                                   reason="gather after AG3b completes")
                    gb16 = p6g.tile([128, D], BF, tag="gb16")
                    nc.vector.tensor_copy(gb16[:], gb[:])
                    for dc in range(8):
                        tp = p6pt.tile([128, 128], BF, tag="btp", bufs=1)
                        nc.tensor.transpose(tp[:], gb16[:, 128 * dc:128 * dc + 128],
                                            identb_sb[:])
                        nc.vector.tensor_copy(bufT[:, dc, 128 * j:128 * j + 128], tp[:])
                h1T = p6.tile([128, 32, CAP], F8)
                w1v = w1l.rearrange("(o p) f -> p o f", p=128)
                for ft in range(32):
                    wt = p6w.tile([128, 8, 128], F8, tag="w1t")
                    nc.sync.dma_start(wt[:], w1v[:, :, 128 * ft:128 * ft + 128])
                    for cs, cw in [(0, 512), (512, 128)]:
                        hp = p6ps.tile([128, 512], FP, tag="hp", bufs=1)
                        for a in range(4):
                            nc.tensor.matmul(hp[:, 0:cw], wt[:, 2 * a:2 * a + 2],
                                             bufT[:, 2 * a:2 * a + 2, cs:cs + cw],
                                             start=(a == 0), stop=(a == 3),
                                             perf_mode=DR)
                        nc.scalar.activation(h1T[:, ft, cs:cs + cw], hp[:, 0:cw],
                                             AF.Gelu, bias=b1l_sb[:, ft:ft + 1],
                                             scale=1.0 / WS)
                eo = p6.tile([128, ST, D], F8)
                w2v = w2l.rearrange("(o p) d -> p o d", p=128)
                for dn in range(2):
                    ops = [p6ps.tile([128, 512], FP, tag=f"op{st}", bufs=1,
                                     name=f"opst{st}")
                           for st in range(ST)]
                    for fkh in range(2):
                        w2t = p6w2.tile([128, 16, 512], F8, tag="w2t")
                        nc.sync.dma_start(w2t[:], w2v[:, 16 * fkh:16 * fkh + 16,
                                                      512 * dn:512 * dn + 512])
                        for st in range(ST):
                            for a in range(8):
                                fa = 8 * fkh + a
                                nc.tensor.matmul(
                                    ops[st][:],
                                    h1T[:, 2 * fa:2 * fa + 2, 128 * st:128 * st + 128],
                                    w2t[:, 2 * a:2 * a + 2],
                                    start=(fa == 0), stop=(fa == 15),
                                    perf_mode=DR)
                    for st in range(ST):
                        nc.vector.scalar_tensor_tensor(
                            eo[:, st, 512 * dn:512 * dn + 512], ops[st][:],
                            inv64_sb[:, 0:1], b2l_sb[:, 512 * dn:512 * dn + 512],
                            AL.mult, AL.add)
                nc.sync.dma_start(ag4_in.rearrange("(s p) d -> p s d", p=128), eo[:])
                if debug:
                    eob = p6.tile([128, ST, D], BF)
                    nc.vector.tensor_copy(eob[:], eo[:])
                    nc.sync.dma_start(dbg["eo"].rearrange("(s p) d -> p s d", p=128),
                                      eob[:])
            with nc.named_scope("cc4"):
                cc4_i = nc.gpsimd.collective_compute(
                    "AllGather", AL.bypass, RG,
                    ins=[ag4_in.opt()], outs=[ag4_out.opt()])

            # ======== P7: combine ========
            with nc.named_scope("P7"), \
                 tc.tile_pool(name="p7", bufs=3) as p7:
                yv = y_out.rearrange("(t p) d -> p t d", p=128)
                for tt in range(4):
                    rows = []
                    for kk in range(2):
                        cr = p7.tile([128, D], F8, tag=f"cr{kk}")
                        nc.vector.memset(cr[:], 0.0)
                        cri = nc.gpsimd.indirect_dma_start(
                            out=cr[:], out_offset=None, in_=ag4_out[:],
                            in_offset=IndirectOffsetOnAxis(ap=cidx[:, tt, kk:kk + 1],
                                                           axis=0),
                            bounds_check=NC * CAP - 1, oob_is_err=False)
                        add_dep_helper(cri.ins, cc4_i.ins, sync=True,
                                       reason="combine after AG4 completes")
                        rows.append(cr)
                    yt = p7.tile([128, D], FP, tag="yt")
                    nc.vector.scalar_tensor_tensor(yt[:], rows[0][:], g12[:, tt, 0:1],
                                                   h_sb[:, tt], AL.mult, AL.add)
                    nc.vector.scalar_tensor_tensor(yt[:], rows[1][:], g12[:, tt, 1:2],
                                                   yt[:], AL.mult, AL.add)
                    nc.sync.dma_start(yv[:, tt], yt[:])

    nc.compile()
    return nc


def _host_inputs(x, ln1_g, ln1_b, w_qkv, w_proj, ln2_g, ln2_b,
                 w_gate, w1, b1, w2, b2):
    x2d = np.asarray(x, np.float32).reshape(N, D)
    w_qkv = np.asarray(w_qkv, np.float32)
    w_proj = np.asarray(w_proj, np.float32)
    ln1_g = np.asarray(ln1_g, np.float32); ln1_b = np.asarray(ln1_b, np.float32)
    ln2_g = np.asarray(ln2_g, np.float32); ln2_b = np.asarray(ln2_b, np.float32)
    w_gate = np.asarray(w_gate, np.float32)
    w1 = np.asarray(w1, np.float32); b1 = np.asarray(b1, np.float32)
    w2 = np.asarray(w2, np.float32); b2 = np.asarray(b2, np.float32)

    pos = np.arange(T, dtype=np.float32)[:, None]
    inv = 1.0 / (10000.0 ** (np.arange(0, HD, 2, dtype=np.float32) / HD))
    ang = pos * inv
    sinN = np.tile(np.sin(ang).T, (1, B))   # [32, N]
    cosN = np.tile(np.cos(ang).T, (1, B))
    Cfull = np.concatenate([cosN] * 4, 0)                  # [128, N]
    Sfull = np.concatenate([sinN, -sinN, sinN, -sinN], 0)  # [128, N] (row content pre-swapped)

    kk = np.arange(128)[:, None]
    qq = np.arange(512)[None, :]
    dmask_np = np.stack([(128 * o + kk <= qq) for o in range(4)]).astype(NPATT)

    c1 = (ln2_g[None, :] * w_gate).sum(1).astype(np.float32)
    c2 = (w_gate @ ln2_b).astype(np.float32)
    gate_fold = (ln2_g[None, :] * w_gate).astype(np.float32)

    def ktiles(a):  # [1024, F] row-major (d = 128*o + p) -> [128, 8, F]
        return np.ascontiguousarray(
            a.reshape(8, 128, a.shape[1]).transpose(1, 0, 2))

    base = {
        "identb": np.eye(128).astype(NPATT),
        "identf": np.eye(128, dtype=np.float32),
        "tri": (np.arange(128)[:, None] <= np.arange(128)[None, :]).astype(ml_dtypes.bfloat16),
        "tri32": (np.arange(32)[:, None] < np.arange(32)[None, :]).astype(np.float32),
        "ones32_128": np.ones((32, 128), ml_dtypes.bfloat16),
        "onesf": np.ones((128, 1), np.float32),
        "onesb": np.ones((128, 1), ml_dtypes.bfloat16),
        "ones_r_f": np.ones((1, 128), np.float32),
        "ln1g": np.ascontiguousarray(ln1_g.reshape(8, 128).T),
        "ln1b": np.ascontiguousarray(ln1_b.reshape(8, 128).T),
        "cc1": np.tile(c1, (128, 1)),
        "cc2": np.tile(c2, (128, 1)),
        "base8": np.tile(np.arange(8, dtype=np.float32) * CAP, (128, 1)),
        "dmask": dmask_np,
        "tid1": np.ascontiguousarray(
            (np.arange(N, dtype=np.int32) + 1).reshape(NT, 128).T),
        "proj_w": ktiles(w_proj.T.copy()).astype(np.float32),
        "gate_w": ktiles(gate_fold.T.copy()),
    }
    sel16_np = np.zeros((16, 8, 128), np.float32)
    for s in range(8):
        sel16_np[2 * s, s, 0:64] = 1.0
        sel16_np[2 * s + 1, s, 64:128] = 1.0
    base["sel16"] = sel16_np

    in_maps = []
    for c in range(NC):
        h0, h1 = 2 * c, 2 * c + 1
        qs = lambda h: w_qkv[192 * h:192 * h + 64]
        ks = lambda h: w_qkv[192 * h + 64:192 * h + 128]
        vs = lambda h: w_qkv[192 * h + 128:192 * h + 192]
        qk = np.concatenate([qs(h0), ks(h0), qs(h1), ks(h1)], 0).T.copy()
        vw = np.concatenate([vs(h0), vs(h1)], 0).T.copy()
        shs = np.zeros((4, NT), np.float32)
        for tt in range(4):
            shs[tt, 4 * c + tt] = 1.0
        ese = np.zeros(8, np.float32)
        ese[c] = 1.0
        m = dict(base)
        m.update({
            "xT": np.ascontiguousarray(
                x2d[SH * c:SH * (c + 1)].T.reshape(8, 128, SH).transpose(1, 0, 2)),
            "qk_w": ktiles(qk).astype(NPATT),
            "v_w": ktiles(vw).astype(NPATT),
            "Ctab": np.ascontiguousarray(Cfull.reshape(128, NT, 128)).astype(NPATT),
            "Stab": np.ascontiguousarray(Sfull.reshape(128, NT, 128)).astype(NPATT),
            "esel": np.tile(ese, (128, 1)),
            "shsel": np.tile(shs[None], (128, 1, 1)),
            "w1l": (WS * ln2_g[:, None] * w1[c]).astype(NPF8),
            "b1l": np.ascontiguousarray(
                (b1[c] + ln2_b @ w1[c]).astype(np.float32).reshape(32, 128).T),
            "w2l": (WS * w2[c]).astype(NPF8),
            "b2l": np.tile(b2[c], (128, 1)).astype(np.float32),
        })
        in_maps.append(m)
    return in_maps


_NC_CACHE = {}


def _get_nc(debug=False):
    key = bool(debug)
    if key not in _NC_CACHE:
        _NC_CACHE[key] = build(debug=debug)
    return _NC_CACHE[key]


def kernel(**inputs):
    debug = bool(inputs.pop("_debug", False))
    want_results = inputs.pop("_want_results", False)
    trace = bool(inputs.pop("_trace", False))
    ncm = _get_nc(debug=debug)
    in_maps = _host_inputs(**inputs)
    res = run_bass_kernel_spmd(ncm, in_maps, core_ids=list(range(NC)), trace=trace)
    y = np.concatenate([res.results[c]["y"] for c in range(NC)], 0).reshape(B, T, D)
    if want_results:
        return y, res
    return y


# revision 32
# speedup vs baseline: 1.2648x; 1.2648x over previous
"""Trainium2 Bass kernel for nn_Block_46471546143558 (moe_routing).

Transformer block: h = x + Attn(LN1(x)); y = h + MoE(LN2(h)).
B=2, T=2048, D=1024, H=16 heads (hd=64), E=8 experts, top-2, cap=640.

Sharding over 8 NeuronCores:
  - attention: head-parallel (2 heads/core); LN/proj/residual token-parallel
    (each core owns a contiguous 512-token shard of the flattened 4096)
  - MoE: expert-parallel (1 expert/core), routing replicated from an
    AllGather of fp32 gate logits
  - collectives: AG(x_ln bf16) -> A2A(attn heads->token shards, fp32
    unnormalized + per-head denominators) -> AG(logits fp32) +
    AG(moe_in fp8) -> AG(expert outputs fp8).

Precision: the routing (top-2 + capacity cumsum) is knife-edge sensitive
(min top2/top3 logit gap ~2e-4), so the path into the logits is kept at
>=fp22 effective precision: probs fp32, PV fp32r, attention payload fp32,
per-head normalization after transfer, proj fp32r, gate matmul fp32.
The MoE value path is insensitive and runs in fp8 (weights pre-scaled
x64 to clear the e4m3 subnormal floor; compensation folded into the
gelu activation scale and the bias add).
"""

import math
import sys

sys.path.insert(0, "/opt/trn_rl_repo")

import ml_dtypes
import numpy as np

import concourse.bass as bass
import concourse.mybir as mybir
import concourse.tile as tile
from concourse import bacc
from concourse.bass import IndirectOffsetOnAxis
from concourse.bass_utils import run_bass_kernel_spmd
from concourse.tile import add_dep_helper

B, T, D, H, E, K = 2, 2048, 1024, 16, 8, 2
HD = D // H  # 64
N = B * T  # 4096
CAP = math.ceil(1.25 * N / E)  # 640
NC = 8
SH = N // NC  # 512
NT = N // 128  # 32
ST = CAP // 128  # 5
F1 = 4 * D  # 4096

FP = mybir.dt.float32
FPR = mybir.dt.float32r
BF = mybir.dt.bfloat16
F8 = mybir.dt.float8e4
F16 = mybir.dt.float16
I32 = mybir.dt.int32
ATT = BF  # attention qkv matmul dtype
NPATT = ml_dtypes.bfloat16
NPF8 = ml_dtypes.float8_e4m3

WS = 64.0  # fp8 weight prescale for w1/w2 (clears e4m3 subnormal floor)

OOB = 1.0e9
AF = mybir.ActivationFunctionType
AL = mybir.AluOpType
DR = mybir.MatmulPerfMode.DoubleRow

PAY = 2 * (HD + 1)  # 130 rows per A2A block: [h0 out 64][h0 den][h1 out 64][h1 den]


def build(debug=False):
    nc = bacc.Bacc("TRN2", target_bir_lowering=False, debug=False,
                   enable_asserts=True, num_devices=NC)

    def din(name, shape, dt):
        return nc.dram_tensor(name, list(shape), dt, kind="ExternalInput").ap()

    xT = din("xT", [128, 8, SH], FP)
    qk_w = din("qk_w", [128, 8, 256], ATT)
    v_w = din("v_w", [128, 8, 128], ATT)
    proj_w = din("proj_w", [128, 8, 1024], FPR)
    gate_w = din("gate_w", [128, 8, 8], FP)
    Ctab = din("Ctab", [128, NT, 128], BF)
    Stab = din("Stab", [128, NT, 128], BF)
    dmask2 = din("dmask2", [2, 128, 1024], BF)
    identb = din("identb", [128, 128], BF)
    identf = din("identf", [128, 128], FP)
    tri = din("tri", [128, 128], BF)
    tri32 = din("tri32", [32, 32], FP)
    ones32_128 = din("ones32_128", [32, 128], BF)
    onesf = din("onesf", [128, 1], FP)
    onesb = din("onesb", [128, 1], BF)
    ones_r_f = din("ones_r_f", [1, 128], FP)
    sel16 = din("sel16", [16, 8, 128], FPR)
    ln1g = din("ln1g", [128, 8], FP)
    ln1b = din("ln1b", [128, 8], FP)
    cc1 = din("cc1", [128, 8], FP)
    cc2 = din("cc2", [128, 8], FP)
    base8 = din("base8", [128, 8], FP)
    esel = din("esel", [128, 8], FP)
    shsel = din("shsel", [128, 4, NT], FP)
    tid1f = din("tid1f", [128, NT], FPR)
    iota640 = din("iota640", [128, 640], FP)
    w1l = din("w1l", [1024, F1], F8)
    b1l = din("b1l", [128, 32], FP)
    w2l = din("w2l", [F1, 1024], F8)
    b2l = din("b2l", [128, 1024], FP)

    y_out = nc.dram_tensor("y", [SH, D], FP, kind="ExternalOutput").ap()
    dbg = {}
    if debug:
        def dout(name, shape, dt=FP):
            return nc.dram_tensor(name, list(shape), dt, kind="ExternalOutput").ap()
        dbg["xln"] = dout("dbg_xln", [128, 8, SH])
        dbg["h"] = dout("dbg_h", [128, 4, D])
        dbg["logits"] = dout("dbg_logits", [128, 4, 8])
        dbg["pos"] = dout("dbg_pos", [128, NT, 8])
        dbg["gidx"] = dout("dbg_gidx", [128, ST], I32)
        dbg["cidx"] = dout("dbg_cidx", [128, 4, 2], I32)
        dbg["g12"] = dout("dbg_g12", [128, 4, 2])
        dbg["eo"] = dout("dbg_eo", [CAP, D], BF)

    ag2_out = nc.dram_tensor("ag2o", [NC * PAY, SH], FPR, kind="Internal").ap()
    ag3b_out = nc.dram_tensor("ag3bo", [N, D], F8, kind="Internal",
                              addr_space="Shared").ap()
    ag4_out = nc.dram_tensor("ag4o", [NC * CAP, D], F8, kind="Internal",
                             addr_space="Shared").ap()
    RG = [list(range(NC))]

    with tile.TileContext(nc) as tc:
        with tc.tile_pool(name="dram", bufs=1, space="DRAM") as dram, \
             tc.tile_pool(name="pers", bufs=1) as pers, \
             tc.tile_pool(name="const", bufs=1) as const:
            ag1_in = dram.tile([128, 8, SH], ATT)
            ag1_out = dram.tile([NC, 128, 8, SH], ATT, addr_space="Shared")
            ag2_in = dram.tile([NC * PAY, SH], FPR)
            ag3a_in = dram.tile([SH, 8], FP)
            ag3a_out = dram.tile([N, 8], FP, addr_space="Shared")
            ag3b_in = dram.tile([SH, D], F8)
            ag4_in = dram.tile([CAP, D], F8)

            qh0 = pers.tile([64, N], ATT)
            qh1 = pers.tile([64, N], ATT)
            kh0 = pers.tile([64, N], ATT)
            kh1 = pers.tile([64, N], ATT)
            v_tm = pers.tile([128, NT, 256], F16)
            xT_sb = pers.tile([128, 8, SH], FP)
            h_sb = pers.tile([128, 4, D], FP)
            m2c = pers.tile([128, 4], FP)
            r2c = pers.tile([128, 4], FP)
            lg_my = pers.tile([128, 4, 8], FP)
            g12 = pers.tile([128, 4, 2], FP)
            cidx = pers.tile([128, 4, 2], I32)
            gidx = pers.tile([128, ST], I32)

            qkw_sb = const.tile([128, 8, 256], ATT)
            vw_sb = const.tile([128, 8, 128], ATT)
            identb_sb = const.tile([128, 128], BF)
            identf_sb = const.tile([128, 128], FP)
            tri_sb = const.tile([128, 128], BF)
            tri32_sb = const.tile([32, 32], FP)
            o32_sb = const.tile([32, 128], BF)
            onesf_sb = const.tile([128, 1], FP)
            onesb_sb = const.tile([128, 1], BF)
            orf_sb = const.tile([1, 128], FP)
            sel16_sb = const.tile([16, 8, 128], FPR)
            ln1g_sb = const.tile([128, 8], FP)
            ln1b_sb = const.tile([128, 8], FP)
            cc1_sb = const.tile([128, 8], FP)
            cc2_sb = const.tile([128, 8], FP)
            base8_sb = const.tile([128, 8], FP)
            esel_sb = const.tile([128, 8], FP)
            shsel_sb = const.tile([128, 4, NT], FP)
            tid1f_sb = const.tile([128, NT], FPR)
            iota_sb = const.tile([128, 640], FP)
            gatew_sb = const.tile([128, 8, 8], FP)
            b1l_sb = const.tile([128, 32], FP)
            b2l_sb = const.tile([128, 1024], FP)
            eps_sb = const.tile([128, 1], FP)
            inv64_sb = const.tile([128, 1], FP)
            nc.vector.memset(eps_sb[:], 1e-5)
            nc.vector.memset(inv64_sb[:], 1.0 / WS)
            dmask2_sb = const.tile([128, 2, 1024], BF)
            nc.sync.dma_start(xT_sb[:], xT[:])
            for sb, dr in [(qkw_sb, qk_w), (vw_sb, v_w), (identb_sb, identb),
                           (identf_sb, identf), (tri_sb, tri), (tri32_sb, tri32),
                           (o32_sb, ones32_128), (onesf_sb, onesf),
                           (onesb_sb, onesb), (orf_sb, ones_r_f),
                           (sel16_sb, sel16), (ln1g_sb, ln1g), (ln1b_sb, ln1b),
                           (cc1_sb, cc1), (cc2_sb, cc2), (base8_sb, base8),
                           (esel_sb, esel), (shsel_sb, shsel), (tid1f_sb, tid1f),
                           (iota_sb, iota640),
                           (gatew_sb, gate_w), (b1l_sb, b1l), (b2l_sb, b2l)]:
                nc.sync.dma_start(sb[:], dr[:])
            for q in range(2):
                nc.sync.dma_start(dmask2_sb[:, q], dmask2[q])

            # ======== P1: LN1 (feature-major) + AG1 ========
            with nc.named_scope("P1"), \
                 tc.tile_pool(name="p1", bufs=1) as p1, \
                 tc.tile_pool(name="p1ps", bufs=2, space="PSUM") as p1ps:
                sq = p1.tile([128, 8, SH], FP)
                nc.scalar.square(sq[:], xT_sb[:])
                msum = p1ps.tile([1, SH], FP, tag="ms")
                ssum = p1ps.tile([1, SH], FP, tag="ss")
                for o in range(8):
                    nc.tensor.matmul(msum[:], onesf_sb[:], xT_sb[:, o],
                                     start=(o == 0), stop=(o == 7))
                for o in range(8):
                    nc.tensor.matmul(ssum[:], onesf_sb[:], sq[:, o],
                                     start=(o == 0), stop=(o == 7))
                mrow = p1.tile([1, SH], FP)
                rrow = p1.tile([1, SH], FP)
                nc.scalar.mul(mrow[:], msum[:], 1.0 / D)
                nc.scalar.mul(rrow[:], ssum[:], 1.0 / D)
                m2row = p1.tile([1, SH], FP)
                nc.vector.tensor_mul(m2row[:], mrow[:], mrow[:])
                nc.vector.tensor_sub(rrow[:], rrow[:], m2row[:])
                nc.scalar.activation(rrow[:], rrow[:], AF.Sqrt, bias=eps_sb[0:1, 0:1])
                nc.vector.reciprocal(rrow[:], rrow[:])
                mb = p1ps.tile([128, SH], FP, tag="bc")
                rb = p1ps.tile([128, SH], FP, tag="bc")
                nc.tensor.matmul(mb[:], orf_sb[:], mrow[:], start=True, stop=True)
                nc.tensor.matmul(rb[:], orf_sb[:], rrow[:], start=True, stop=True)
                xln = p1.tile([128, 8, SH], ATT)
                for o in range(8):
                    t1 = p1.tile([128, SH], FP, tag="t1")
                    nc.vector.tensor_sub(t1[:], xT_sb[:, o], mb[:])
                    nc.vector.tensor_mul(t1[:], t1[:], rb[:])
                    nc.vector.tensor_scalar(xln[:, o], t1[:],
                                            ln1g_sb[:, o:o + 1], ln1b_sb[:, o:o + 1],
                                            AL.mult, AL.add)
                nc.sync.dma_start(ag1_in[:], xln[:])
                if debug:
                    xlnf = p1.tile([128, 8, SH], FP)
                    nc.vector.tensor_copy(xlnf[:], xln[:])
                    nc.sync.dma_start(dbg["xln"][:], xlnf[:])
            with nc.named_scope("cc1"):
                nc.gpsimd.collective_compute("AllGather", AL.bypass, RG,
                                             ins=[ag1_in.opt()], outs=[ag1_out.opt()])

            # ======== P2: QKV + RoPE + V token-major ========
            with nc.named_scope("P2"), \
                 tc.tile_pool(name="p2", bufs=2) as p2, \
                 tc.tile_pool(name="p2c", bufs=1) as p2c, \
                 tc.tile_pool(name="p2ps", bufs=2, space="PSUM") as p2ps, \
                 tc.tile_pool(name="p2v", bufs=2, space="PSUM") as p2v:
                ct_sb = p2c.tile([128, NT, 128], BF)
                st_sb = p2c.tile([128, NT, 128], BF)
                nc.sync.dma_start(ct_sb[:], Ctab[:])
                nc.sync.dma_start(st_sb[:], Stab[:])
                nc.vector.memset(v_tm[:], 0.0)
                nc.vector.memset(v_tm[:, :, 0:1], 1.0)
                nc.vector.memset(v_tm[:, :, 128:129], 1.0)
                v_tm4 = v_tm.rearrange("p t (a b) -> p t a b", a=2)
                for s in range(NC):
                    xl = p2.tile([128, 8, SH], ATT, tag="xl")
                    nc.sync.dma_start(xl[:], ag1_out[s])
                    for mt in range(2):
                        qdst = qh0 if mt == 0 else qh1
                        kdst = kh0 if mt == 0 else kh1
                        ps = p2ps.tile([128, SH], FP, tag=f"qk{mt}", bufs=1)
                        for o in range(8):
                            nc.tensor.matmul(ps[:], qkw_sb[:, o, 128 * mt:128 * mt + 128],
                                             xl[:, o], start=(o == 0), stop=(o == 7))
                        Cv = ct_sb.rearrange("p t f -> p (t f)")[:, SH * s:SH * (s + 1)]
                        Sv = st_sb.rearrange("p t f -> p (t f)")[:, SH * s:SH * (s + 1)]
                        psb = p2.tile([128, SH], BF, tag="psb")
                        nc.vector.tensor_copy(psb[:], ps[:])
                        tmp = p2.tile([128, SH], BF, tag="tmp")
                        acc = p2.tile([128, SH], BF, tag="acc")
                        nc.vector.tensor_mul(tmp[32:64], psb[0:32], Sv[0:32])
                        nc.vector.tensor_mul(tmp[0:32], psb[32:64], Sv[32:64])
                        nc.vector.tensor_mul(tmp[96:128], psb[64:96], Sv[64:96])
                        nc.vector.tensor_mul(tmp[64:96], psb[96:128], Sv[96:128])
                        nc.vector.tensor_mul(acc[:], psb[:], Cv[:])
                        nc.vector.tensor_add(qdst[:, SH * s:SH * (s + 1)],
                                             acc[0:64], tmp[0:64])
                        nc.vector.tensor_add(kdst[:, SH * s:SH * (s + 1)],
                                             acc[64:128], tmp[64:128])
                    for tt in range(4):
                        vps = p2v.tile([128, 128], FP, tag="v")
                        for o in range(8):
                            nc.tensor.matmul(vps[:], xl[:, o, 128 * tt:128 * tt + 128],
                                             vw_sb[:, o], start=(o == 0), stop=(o == 7))
                        g = 4 * s + tt
                        nc.vector.tensor_copy(
                            v_tm4[:, g, :, 64:128],
                            vps.rearrange("p (a b) -> p a b", a=2))

            # ======== P3: scores + softmax + PV (unnormalized); A2A ========
            with nc.named_scope("P3"), \
                 tc.tile_pool(name="p3", bufs=3) as p3, \
                 tc.tile_pool(name="p3ps", bufs=2, space="PSUM") as p3ps, \
                 tc.tile_pool(name="p3po", bufs=2, space="PSUM") as p3po:
                a2v = ag2_in.rearrange("(k r) t -> k r t", r=PAY)
                for hh in range(2):
                    qh = qh0 if hh == 0 else qh1
                    kh = kh0 if hh == 0 else kh1
                    for qb in range(8):
                        seq, qbl = qb // 4, qb % 4
                        kts = list(range(16 * seq, 16 * seq + 4 * qbl + 4))
                        npair = len(kts) // 2
                        ov = p3po.tile([128, 512], FP, tag="ov")
                        for i in range(npair):
                            kt0, kt1 = kts[2 * i], kts[2 * i + 1]
                            sc = p3ps.tile([128, 1024], FP, tag="sc")
                            nc.tensor.matmul(sc[:, 0:512],
                                             kh[:, 128 * kt0:128 * kt0 + 128],
                                             qh[:, 512 * qb:512 * (qb + 1)],
                                             start=True, stop=True)
                            nc.tensor.matmul(sc[:, 512:1024],
                                             kh[:, 128 * kt1:128 * kt1 + 128],
                                             qh[:, 512 * qb:512 * (qb + 1)],
                                             start=True, stop=True)
                            ex = p3.tile([128, 1024], F16, tag="ex")
                            nc.scalar.activation(ex[:], sc[:], AF.Exp,
                                                 scale=1.0 / math.sqrt(HD))
                            loc = 2 * i - 4 * qbl
                            if loc >= 0:
                                nc.vector.tensor_mul(ex[:], ex[:],
                                                     dmask2_sb[:, loc // 2])
                            nc.tensor.matmul(ov[:],
                                             v_tm[:, kt0, 128 * hh:128 * hh + 128],
                                             ex[:, 0:512],
                                             start=(i == 0), stop=False)
                            nc.tensor.matmul(ov[:],
                                             v_tm[:, kt1, 128 * hh:128 * hh + 128],
                                             ex[:, 512:1024],
                                             start=False, stop=(i == npair - 1))
                        av = p3.tile([65, 512], FPR, tag="av")
                        nc.vector.tensor_copy(av[0:64], ov[64:128])
                        nc.vector.tensor_copy(av[64:65], ov[0:1])
                        r0 = 65 * hh
                        nc.sync.dma_start(a2v[qb, r0:r0 + 65], av[:])
            with nc.named_scope("cc2"):
                nc.gpsimd.collective_compute("AllToAll", AL.bypass, RG,
                                             ins=[ag2_in.opt()], outs=[ag2_out.opt()])

            # ======== P4: normalize + proj + h + LN2 + logits + moe_in ========
            with nc.named_scope("P4"), \
                 tc.tile_pool(name="p4", bufs=1) as p4, \
                 tc.tile_pool(name="p4ps", bufs=2, space="PSUM") as p4ps, \
                 tc.tile_pool(name="p4pt", bufs=1, space="PSUM") as p4pt:
                o2v = ag2_out.rearrange("(k r) t -> k r t", r=PAY)
                o2d = ag2_out.rearrange("(k j s) t -> k j s t", j=2, s=HD + 1)
                myc = p4.tile([128, 8, SH], FPR)
                nc.sync.dma_start(myc[0:64], o2v[:, 0:64].rearrange("k r t -> r k t"))
                nc.sync.dma_start(myc[64:128], o2v[:, 65:129].rearrange("k r t -> r k t"))
                den16 = p4.tile([16, SH], FPR)
                nc.sync.dma_start(den16[:], o2d[:, :, 64].rearrange("k j t -> (k j) t"))
                with nc.allow_low_precision(reason="fp32r recip feeds fp32r matmul"):
                    nc.vector.reciprocal(den16[:], den16[:])
                hT = p4.tile([128, 8, SH], FP)
                pw = p4.tile([128, 8, 1024], FPR)
                nc.sync.dma_start(pw[:], proj_w[:])
                for ki in range(8):
                    rbp = p4pt.tile([128, SH], FP, tag="rb")
                    nc.tensor.matmul(rbp[:], sel16_sb[:, ki],
                                     den16[:], start=True, stop=True)
                    nc.vector.tensor_mul(myc[:, ki], myc[:, ki], rbp[:])
                for do in range(8):
                    pp = p4ps.tile([128, SH], FP, tag="pp")
                    for ki in range(8):
                        nc.tensor.matmul(pp[:], pw[:, ki, 128 * do:128 * do + 128],
                                         myc[:, ki],
                                         start=(ki == 0), stop=(ki == 7))
                    nc.vector.tensor_add(hT[:, do], pp[:], xT_sb[:, do])
                    for tt in range(4):
                        tp = p4pt.tile([128, 128], FP, tag="tp", bufs=2)
                        nc.tensor.transpose(tp[:], hT[:, do, 128 * tt:128 * tt + 128],
                                            identf_sb[:])
                        nc.vector.tensor_copy(h_sb[:, tt, 128 * do:128 * do + 128], tp[:])
                scr = p4.tile([128, D], FP, tag="scr")
                sqc = p4.tile([128, 4], FP, tag="sqc")
                for tt in range(4):
                    nc.vector.tensor_reduce(m2c[:, tt:tt + 1], h_sb[:, tt],
                                            axis=mybir.AxisListType.X, op=AL.add)
                    nc.scalar.activation(scr[:], h_sb[:, tt], AF.Square,
                                         accum_out=sqc[:, tt:tt + 1])
                nc.vector.tensor_scalar_mul(m2c[:], m2c[:], 1.0 / D)
                nc.vector.tensor_scalar_mul(sqc[:], sqc[:], 1.0 / D)
                vv = p4.tile([128, 4], FP, tag="vv")
                nc.vector.tensor_mul(vv[:], m2c[:], m2c[:])
                nc.vector.tensor_sub(vv[:], sqc[:], vv[:])
                nc.scalar.activation(vv[:], vv[:], AF.Sqrt, bias=eps_sb[:, 0:1])
                nc.vector.reciprocal(r2c[:], vv[:])
                lps = p4pt.tile([8, SH], FP, tag="lps")
                for ki in range(8):
                    nc.tensor.matmul(lps[:], gatew_sb[:, ki], hT[:, ki],
                                     start=(ki == 0), stop=(ki == 7))
                lsb = p4.tile([8, SH], FP, tag="lsb")
                nc.vector.tensor_copy(lsb[:], lps[:])
                rm = p4.tile([128, 4], FP, tag="rm")
                nc.vector.tensor_mul(rm[:], r2c[:], m2c[:])
                t8 = p4.tile([128, 8], FP, tag="t8")
                for tt in range(4):
                    ltp = p4pt.tile([128, 8], FP, tag="ltp")
                    nc.tensor.transpose(ltp[:], lsb[0:8, 128 * tt:128 * tt + 128],
                                        identf_sb[0:8, 0:8])
                    nc.vector.tensor_scalar_mul(lg_my[:, tt], ltp[:], r2c[:, tt:tt + 1])
                    nc.vector.tensor_scalar_mul(t8[:], cc1_sb[:], rm[:, tt:tt + 1])
                    nc.vector.tensor_sub(lg_my[:, tt], lg_my[:, tt], t8[:])
                    nc.vector.tensor_add(lg_my[:, tt], lg_my[:, tt], cc2_sb[:])
                nc.sync.dma_start(ag3a_in.rearrange("(t p) e -> p t e", p=128), lg_my[:])
                moein = p4.tile([128, 4, D], F8)
                for tt in range(4):
                    nc.vector.tensor_scalar(moein[:, tt], h_sb[:, tt],
                                            m2c[:, tt:tt + 1], r2c[:, tt:tt + 1],
                                            AL.subtract, AL.mult)
                nc.sync.dma_start(ag3b_in.rearrange("(t p) d -> p t d", p=128), moein[:])
                if debug:
                    nc.sync.dma_start(dbg["h"][:], h_sb[:])
                    nc.sync.dma_start(dbg["logits"][:], lg_my[:])
            with nc.named_scope("cc3a"):
                cc3a_i = nc.gpsimd.collective_compute(
                    "AllGather", AL.bypass, RG,
                    ins=[ag3a_in.opt()], outs=[ag3a_out.opt()])
            with nc.named_scope("cc3b"):
                cc3b_i = nc.gpsimd.collective_compute(
                    "AllGather", AL.bypass, RG,
                    ins=[ag3b_in.opt()], outs=[ag3b_out.opt()])
                add_dep_helper(cc3b_i.ins, cc3a_i.ins, sync=True,
                               reason="AG3a (routing) before AG3b on CC core")

            # ======== P5: replicated routing ========
            with nc.named_scope("P5"), \
                 tc.tile_pool(name="p5", bufs=2) as p5, \
                 tc.tile_pool(name="p5ps", bufs=2, space="PSUM") as p5ps:
                lg4 = p5.tile([128, NC, 4, 8], FP)
                nc.sync.dma_start(lg4[:], ag3a_out.rearrange("(r t p) e -> p r t e",
                                                             p=128, t=4))
                lg_all = lg4.rearrange("p r t e -> p (r t) e")
                mask = p5.tile([128, NT, 8], BF)
                v8 = p5.tile([128, 8], FP, tag="v8")
                m1 = p5.tile([128, NT, 1], FP, tag="m1")
                mask0 = p5.tile([128, NT, 8], FP, tag="mask0")
                lg2 = p5.tile([128, NT, 8], FP, tag="lg2")
                nc.vector.tensor_reduce(m1[:], lg_all[:],
                                        axis=mybir.AxisListType.X, op=AL.max)
                nc.vector.tensor_tensor(mask0[:], lg_all[:],
                                        m1[:].to_broadcast([128, NT, 8]), AL.is_ge)
                nc.vector.tensor_scalar(lg2[:], mask0[:], -OOB, None, AL.mult)
                nc.vector.tensor_add(lg2[:], lg2[:], lg_all[:])
                nc.vector.tensor_reduce(m1[:], lg2[:],
                                        axis=mybir.AxisListType.X, op=AL.max)
                nc.vector.tensor_tensor(mask[:], lg_all[:],
                                        m1[:].to_broadcast([128, NT, 8]), AL.is_ge)
                trow_ps = p5ps.tile([1, NT * 8], FP, tag="bc")
                nc.tensor.matmul(trow_ps[:], onesb_sb[:],
                                 mask.rearrange("p t e -> p (t e)"), start=True, stop=True)
                trow = p5.tile([1, NT * 8], BF, tag="trow")
                nc.vector.tensor_copy(trow[:], trow_ps[:])
                totd = dram.tile([NT, 8], BF)
                nc.sync.dma_start(totd.rearrange("t e -> (t e)")[None, :], trow[:])
                tot32 = p5.tile([32, 8], BF, tag="tot32")
                nc.sync.dma_start(tot32[:], totd[:])
                rhs2 = p5.tile([32, NT, 8], BF, tag="rhs2")
                nc.vector.tensor_tensor(rhs2[:],
                                        tot32[:, None, :].to_broadcast([32, NT, 8]),
                                        tri32_sb[:, :, None].to_broadcast([32, NT, 8]),
                                        AL.mult)
                pos_ps = p5ps.tile([128, NT * 8], FP, tag="pos")
                nc.tensor.matmul(pos_ps[:], tri_sb[:], mask.rearrange("p t e -> p (t e)"),
                                 start=True, stop=False)
                nc.tensor.matmul(pos_ps[:], o32_sb[:], rhs2.rearrange("p t e -> p (t e)"),
                                 start=False, stop=True)
                pos = p5.tile([128, NT, 8], FP)
                nc.vector.tensor_copy(pos[:], pos_ps.rearrange("p (t e) -> p t e", e=8))
                if debug:
                    nc.sync.dma_start(dbg["pos"][:], pos[:])
                # dispatch offsets for my expert
                pe = p5.tile([128, NT], FP, tag="pe")
                me = p5.tile([128, NT], FP, tag="me")
                scr8 = p5.tile([128, 8], FP, tag="scr8")
                eb = p5.tile([128, NT, 8], FP, tag="eb")
                esb = esel_sb[:, None, :].to_broadcast([128, NT, 8])
                nc.vector.tensor_tensor(eb[:], pos[:], esb, AL.mult)
                nc.vector.tensor_reduce(pe[:, :, None], eb[:],
                                        axis=mybir.AxisListType.X, op=AL.add)
                nc.vector.tensor_tensor(eb[:], mask[:], esb, AL.mult)
                nc.vector.tensor_reduce(me[:, :, None], eb[:],
                                        axis=mybir.AxisListType.X, op=AL.add)
                offf = p5.tile([128, NT], FP, tag="offf")
                nc.vector.tensor_scalar(offf[:], me[:], -OOB, OOB, AL.mult, AL.add)
                nc.vector.tensor_add(offf[:], offf[:], pe[:])
                nc.vector.tensor_scalar_add(offf[:], offf[:], -1.0)
                accA = p5ps.tile([1, 512], FP, tag="accA", bufs=1)
                accB = p5ps.tile([1, 128], FP, tag="accB", bufs=1)
                for g in range(NT):
                    eq = p5.tile([128, 640], FPR, tag="eq", bufs=3)
                    with nc.allow_low_precision(reason="fp32r one-hot exact ints"):
                        nc.vector.tensor_scalar(eq[:], iota_sb[:],
                                                offf[:, g:g + 1], None, AL.is_equal)
                    nc.tensor.matmul(accA[:], tid1f_sb[:, g:g + 1], eq[:, 0:512],
                                     start=(g == 0), stop=(g == NT - 1))
                    nc.tensor.matmul(accB[:], tid1f_sb[:, g:g + 1], eq[:, 512:640],
                                     start=(g == 0), stop=(g == NT - 1))
                accS = p5.tile([1, 640], FP, tag="accS")
                nc.vector.tensor_copy(accS[:, 0:512], accA[:])
                nc.vector.tensor_copy(accS[:, 512:640], accB[:])
                gidxf = p5.tile([128, ST], FP, tag="gidxf")
                for j in range(ST):
                    tpj = p5ps.tile([128, 1], FP, tag="tpj", bufs=2)
                    nc.tensor.transpose(tpj[:], accS[0:1, 128 * j:128 * j + 128],
                                        identf_sb[0:1, 0:1])
                    nc.vector.tensor_copy(gidxf[:, j:j + 1], tpj[:])
                nc.vector.tensor_scalar_add(gidxf[:], gidxf[:], -1.0)
                nc.vector.tensor_copy(gidx[:], gidxf[:])
                if debug:
                    nc.sync.dma_start(dbg["gidx"][:], gidx[:])
                # my combine indices + gate values
                myp = p5.tile([128, 4, 8], FP, tag="myp")
                tmpb = p5.tile([128, NT, 8], FP, tag="tmpb")
                for tt in range(4):
                    nc.vector.tensor_tensor(tmpb[:], pos[:],
                                            shsel_sb[:, tt, :, None].to_broadcast(
                                                [128, NT, 8]), AL.mult)
                    nc.vector.tensor_reduce(myp[:, tt, :, None],
                                            tmpb.rearrange("p t e -> p e t"),
                                            axis=mybir.AxisListType.X, op=AL.add)
                oh0 = p5.tile([128, 8], FP, tag="oh0")
                oh1 = p5.tile([128, 8], FP, tag="oh1")
                ex8 = p5.tile([128, 8], FP, tag="ex8")
                den8 = p5.tile([128, 1], FP, tag="den8")
                v12 = p5.tile([128, 2], FP, tag="v12")
                fl = p5.tile([128, 1], FP, tag="fl")
                sl = p5.tile([128, 1], FP, tag="sl")
                kf = p5.tile([128, 1], FP, tag="kf")
                tof = p5.tile([128, 1], FP, tag="tof")
                cidf = p5.tile([128, 4, 2], FP, tag="cidf")
                fb = p5.tile([128, 8], FP, tag="fb")
                for tt in range(4):
                    nc.vector.max(v8[:], lg_my[:, tt])
                    nc.vector.tensor_copy(v12[:], v8[:, 0:2])
                    nc.scalar.activation(ex8[:], lg_my[:, tt], AF.Exp, accum_out=den8[:])
                    nc.vector.reciprocal(den8[:], den8[:])
                    nc.scalar.activation(g12[:, tt], v12[:], AF.Exp)
                    nc.vector.tensor_scalar_mul(g12[:, tt], g12[:, tt], den8[:])
                    nc.vector.tensor_add(fb[:], myp[:, tt], base8_sb[:])
                    nc.vector.tensor_scalar(oh0[:], lg_my[:, tt], v8[:, 0:1], None, AL.is_ge)
                    nc.vector.tensor_scalar(oh1[:], lg_my[:, tt], v8[:, 1:2], None, AL.is_ge)
                    nc.vector.tensor_sub(oh1[:], oh1[:], oh0[:])
                    for kk, oh in ((0, oh0), (1, oh1)):
                        nc.vector.tensor_mul(scr8[:], myp[:, tt], oh[:])
                        nc.vector.tensor_reduce(sl[:], scr8[:],
                                                axis=mybir.AxisListType.X, op=AL.add)
                        nc.vector.tensor_mul(scr8[:], fb[:], oh[:])
                        nc.vector.tensor_reduce(fl[:], scr8[:],
                                                axis=mybir.AxisListType.X, op=AL.add)
                        nc.vector.tensor_scalar(kf[:], sl[:], CAP + 0.5, None, AL.is_le)
                        nc.vector.tensor_scalar(tof[:], kf[:], -OOB, OOB, AL.mult, AL.add)
                        nc.vector.scalar_tensor_tensor(cidf[:, tt, kk:kk + 1], fl[:],
                                                       kf[:, 0:1], tof[:],
                                                       AL.mult, AL.add)
                nc.vector.tensor_scalar_add(cidf[:], cidf[:], -1.0)
                nc.vector.tensor_copy(cidx[:], cidf[:])
                if debug:
                    nc.sync.dma_start(dbg["cidx"][:], cidx[:])
                    nc.sync.dma_start(dbg["g12"][:], g12[:])

            # ======== P6: expert MLP (fp8, DoubleRow); AG4 ========
            with nc.named_scope("P6"), \
                 tc.tile_pool(name="p6", bufs=1) as p6, \
                 tc.tile_pool(name="p6g", bufs=2) as p6g, \
                 tc.tile_pool(name="p6w", bufs=2) as p6w, \
                 tc.tile_pool(name="p6w2", bufs=1) as p6w2, \
                 tc.tile_pool(name="p6ps", bufs=2, space="PSUM") as p6ps, \
                 tc.tile_pool(name="p6pt", bufs=2, space="PSUM") as p6pt:
                bufT = p6.tile([128, 8, CAP], F8)
                for j in range(ST):
                    gb = p6g.tile([128, D], F8, tag="gb")
                    nc.vector.memset(gb[:], 0.0)
                    gbi = nc.gpsimd.indirect_dma_start(
                        out=gb[:], out_offset=None, in_=ag3b_out[:],
                        in_offset=IndirectOffsetOnAxis(ap=gidx[:, j:j + 1], axis=0),
                        bounds_check=N - 1, oob_is_err=False)
                    add_dep_helper(gbi.ins, cc3b_i.ins, sync=True,
                                   reason="gather after AG3b completes")
                    gb16 = p6g.tile([128, D], BF, tag="gb16")
                    nc.vector.tensor_copy(gb16[:], gb[:])
                    for dc in range(8):
                        tp = p6pt.tile([128, 128], BF, tag="btp", bufs=1)
                        nc.tensor.transpose(tp[:], gb16[:, 128 * dc:128 * dc + 128],
                                            identb_sb[:])
                        nc.vector.tensor_copy(bufT[:, dc, 128 * j:128 * j + 128], tp[:])
                h1T = p6.tile([128, 32, CAP], F8)
                w1v = w1l.rearrange("(o p) f -> p o f", p=128)
                for ft in range(32):
                    wt = p6w.tile([128, 8, 128], F8, tag="w1t")
                    nc.sync.dma_start(wt[:], w1v[:, :, 128 * ft:128 * ft + 128])
                    for cs, cw in [(0, 512), (512, 128)]:
                        hp = p6ps.tile([128, 512], FP, tag="hp", bufs=1)
                        for a in range(4):
                            nc.tensor.matmul(hp[:, 0:cw], wt[:, 2 * a:2 * a + 2],
                                             bufT[:, 2 * a:2 * a + 2, cs:cs + cw],
                                             start=(a == 0), stop=(a == 3),
                                             perf_mode=DR)
                        nc.scalar.activation(h1T[:, ft, cs:cs + cw], hp[:, 0:cw],
                                             AF.Gelu, bias=b1l_sb[:, ft:ft + 1],
                                             scale=1.0 / WS)
                eo = p6.tile([128, ST, D], F8)
                w2v = w2l.rearrange("(o p) d -> p o d", p=128)
                for dn in range(2):
                    ops = [p6ps.tile([128, 512], FP, tag=f"op{st}", bufs=1,
                                     name=f"opst{st}")
                           for st in range(ST)]
                    for fkh in range(2):
                        w2t = p6w2.tile([128, 16, 512], F8, tag="w2t")
                        nc.sync.dma_start(w2t[:], w2v[:, 16 * fkh:16 * fkh + 16,
                                                      512 * dn:512 * dn + 512])
                        for st in range(ST):
                            for a in range(8):
                                fa = 8 * fkh + a
                                nc.tensor.matmul(
                                    ops[st][:],
                                    h1T[:, 2 * fa:2 * fa + 2, 128 * st:128 * st + 128],
                                    w2t[:, 2 * a:2 * a + 2],
                                    start=(fa == 0), stop=(fa == 15),
                                    perf_mode=DR)
                    for st in range(ST):
                        nc.vector.scalar_tensor_tensor(
                            eo[:, st, 512 * dn:512 * dn + 512], ops[st][:],
                            inv64_sb[:, 0:1], b2l_sb[:, 512 * dn:512 * dn + 512],
                            AL.mult, AL.add)
                nc.sync.dma_start(ag4_in.rearrange("(s p) d -> p s d", p=128), eo[:])
                if debug:
                    eob = p6.tile([128, ST, D], BF)
                    nc.vector.tensor_copy(eob[:], eo[:])
                    nc.sync.dma_start(dbg["eo"].rearrange("(s p) d -> p s d", p=128),
                                      eob[:])
            with nc.named_scope("cc4"):
                cc4_i = nc.gpsimd.collective_compute(
                    "AllGather", AL.bypass, RG,
                    ins=[ag4_in.opt()], outs=[ag4_out.opt()])

            # ======== P7: combine ========
            with nc.named_scope("P7"), \
                 tc.tile_pool(name="p7", bufs=3) as p7:
                yv = y_out.rearrange("(t p) d -> p t d", p=128)
                for tt in range(4):
                    rows = []
                    for kk in range(2):
                        cr = p7.tile([128, D], F8, tag=f"cr{kk}")
                        nc.vector.memset(cr[:], 0.0)
                        cri = nc.gpsimd.indirect_dma_start(
                            out=cr[:], out_offset=None, in_=ag4_out[:],
                            in_offset=IndirectOffsetOnAxis(ap=cidx[:, tt, kk:kk + 1],
                                                           axis=0),
                            bounds_check=NC * CAP - 1, oob_is_err=False)
                        add_dep_helper(cri.ins, cc4_i.ins, sync=True,
                                       reason="combine after AG4 completes")
                        rows.append(cr)
                    yt = p7.tile([128, D], FP, tag="yt")
                    nc.vector.scalar_tensor_tensor(yt[:], rows[0][:], g12[:, tt, 0:1],
                                                   h_sb[:, tt], AL.mult, AL.add)
                    nc.vector.scalar_tensor_tensor(yt[:], rows[1][:], g12[:, tt, 1:2],
                                                   yt[:], AL.mult, AL.add)
                    nc.sync.dma_start(yv[:, tt], yt[:])

    nc.compile()
    return nc


def _host_inputs(x, ln1_g, ln1_b, w_qkv, w_proj, ln2_g, ln2_b,
                 w_gate, w1, b1, w2, b2):
    x2d = np.asarray(x, np.float32).reshape(N, D)
    w_qkv = np.asarray(w_qkv, np.float32)
    w_proj = np.asarray(w_proj, np.float32)
    ln1_g = np.asarray(ln1_g, np.float32); ln1_b = np.asarray(ln1_b, np.float32)
    ln2_g = np.asarray(ln2_g, np.float32); ln2_b = np.asarray(ln2_b, np.float32)
    w_gate = np.asarray(w_gate, np.float32)
    w1 = np.asarray(w1, np.float32); b1 = np.asarray(b1, np.float32)
    w2 = np.asarray(w2, np.float32); b2 = np.asarray(b2, np.float32)

    pos = np.arange(T, dtype=np.float32)[:, None]
    inv = 1.0 / (10000.0 ** (np.arange(0, HD, 2, dtype=np.float32) / HD))
    ang = pos * inv
    sinN = np.tile(np.sin(ang).T, (1, B))   # [32, N]
    cosN = np.tile(np.cos(ang).T, (1, B))
    Cfull = np.concatenate([cosN] * 4, 0)                  # [128, N]
    Sfull = np.concatenate([sinN, -sinN, sinN, -sinN], 0)  # [128, N] (row content pre-swapped)

    kk = np.arange(128)[:, None]
    qq = np.arange(512)[None, :]
    dmask_np = np.stack([(128 * o + kk <= qq) for o in range(4)]).astype(NPATT)

    c1 = (ln2_g[None, :] * w_gate).sum(1).astype(np.float32)
    c2 = (w_gate @ ln2_b).astype(np.float32)
    gate_fold = (ln2_g[None, :] * w_gate).astype(np.float32)

    def ktiles(a):  # [1024, F] row-major (d = 128*o + p) -> [128, 8, F]
        return np.ascontiguousarray(
            a.reshape(8, 128, a.shape[1]).transpose(1, 0, 2))

    base = {
        "identb": np.eye(128).astype(NPATT),
        "identf": np.eye(128, dtype=np.float32),
        "tri": (np.arange(128)[:, None] <= np.arange(128)[None, :]).astype(ml_dtypes.bfloat16),
        "tri32": (np.arange(32)[:, None] < np.arange(32)[None, :]).astype(np.float32),
        "ones32_128": np.ones((32, 128), ml_dtypes.bfloat16),
        "onesf": np.ones((128, 1), np.float32),
        "onesb": np.ones((128, 1), ml_dtypes.bfloat16),
        "ones_r_f": np.ones((1, 128), np.float32),
        "ln1g": np.ascontiguousarray(ln1_g.reshape(8, 128).T),
        "ln1b": np.ascontiguousarray(ln1_b.reshape(8, 128).T),
        "cc1": np.tile(c1, (128, 1)),
        "cc2": np.tile(c2, (128, 1)),
        "base8": np.tile(np.arange(8, dtype=np.float32) * CAP, (128, 1)),
        "dmask2": np.stack([
            np.concatenate([dmask_np[0], dmask_np[1]], -1),
            np.concatenate([dmask_np[2], dmask_np[3]], -1)]),
        "tid1f": np.ascontiguousarray(
            (np.arange(N, dtype=np.float32) + 1).reshape(NT, 128).T),
        "iota640": np.tile(np.arange(640, dtype=np.float32), (128, 1)),
        "proj_w": ktiles(w_proj.T.copy()).astype(np.float32),
        "gate_w": ktiles(gate_fold.T.copy()),
    }
    sel16_np = np.zeros((16, 8, 128), np.float32)
    for s in range(8):
        sel16_np[2 * s, s, 0:64] = 1.0
        sel16_np[2 * s + 1, s, 64:128] = 1.0
    base["sel16"] = sel16_np

    in_maps = []
    for c in range(NC):
        h0, h1 = 2 * c, 2 * c + 1
        qs = lambda h: w_qkv[192 * h:192 * h + 64]
        ks = lambda h: w_qkv[192 * h + 64:192 * h + 128]
        vs = lambda h: w_qkv[192 * h + 128:192 * h + 192]
        qk = np.concatenate([qs(h0), ks(h0), qs(h1), ks(h1)], 0).T.copy()
        vw = np.concatenate([vs(h0), vs(h1)], 0).T.copy()
        shs = np.zeros((4, NT), np.float32)
        for tt in range(4):
            shs[tt, 4 * c + tt] = 1.0
        ese = np.zeros(8, np.float32)
        ese[c] = 1.0
        m = dict(base)
        m.update({
            "xT": np.ascontiguousarray(
                x2d[SH * c:SH * (c + 1)].T.reshape(8, 128, SH).transpose(1, 0, 2)),
            "qk_w": ktiles(qk).astype(NPATT),
            "v_w": ktiles(vw).astype(NPATT),
            "Ctab": np.ascontiguousarray(Cfull.reshape(128, NT, 128)).astype(NPATT),
            "Stab": np.ascontiguousarray(Sfull.reshape(128, NT, 128)).astype(NPATT),
            "esel": np.tile(ese, (128, 1)),
            "shsel": np.tile(shs[None], (128, 1, 1)),
            "w1l": (WS * ln2_g[:, None] * w1[c]).astype(NPF8),
            "b1l": np.ascontiguousarray(
                (b1[c] + ln2_b @ w1[c]).astype(np.float32).reshape(32, 128).T),
            "w2l": (WS * w2[c]).astype(NPF8),
            "b2l": np.tile(b2[c], (128, 1)).astype(np.float32),
        })
        in_maps.append(m)
    return in_maps


_NC_CACHE = {}


def _get_nc(debug=False):
    key = bool(debug)
    if key not in _NC_CACHE:
        _NC_CACHE[key] = build(debug=debug)
    return _NC_CACHE[key]


def kernel(**inputs):
    debug = bool(inputs.pop("_debug", False))
    want_results = inputs.pop("_want_results", False)
    trace = bool(inputs.pop("_trace", False))
    ncm = _get_nc(debug=debug)
    in_maps = _host_inputs(**inputs)
    res = run_bass_kernel_spmd(ncm, in_maps, core_ids=list(range(NC)), trace=trace)
    y = np.concatenate([res.results[c]["y"] for c in range(NC)], 0).reshape(B, T, D)
    if want_results:
        return y, res
    return y
